# revision 11
# baseline (speedup 1.0000x reference)
"""MultiHeadAttention Bass kernel for 8 Trainium2 NeuronCores.

Reference computation (B=4, S=2048, D=1024, H=16, DK=64):
    qh, kh, vh = proj(q), proj(k), proj(v)          # all use SAME Wq/bq
    attn = softmax((qh/8) @ kh^T)                   # [B,H,S,S], graded output
    out  = attn @ vh  -> concat -> @ Wfc + bfc
    x    = layernorm(q + out) * gamma + beta        # graded output

Sharding: data-parallel over (batch, query-half). Core c handles batch
c//2, query rows [(c%2)*1024, +1024).  K/V projections are duplicated
across the two cores of a batch; no collectives needed.

Per-core dataflow (fp32r/fp16 matmul operands, fp32 accumulation):
  Phase 1: PE-transpose q/k/v row-blocks -> X^T; project to
           qhT [cols, qrows] (x0.125), khT [cols, krows],
           vh  [krows, heads, 65] (col 64 = ones so the PV matmul also
           produces the softmax denominator).  Biases are added via an
           extra K=1 matmul against a ones row (DVE cannot broadcast
           across partitions).
  Phase 2: per (head-pair, q-half): logitsT = khT-chunk @ qhT
           (k on partitions); exp on ACT (fp16; no max-subtraction:
           |logits| <~ 4); PV accumulates out^T [65, q] over k-chunks,
           row 64 = sum(exp) = den; attn is written in natural layout
           by PE-transposing the exp chunks and multiplying by 1/den
           during the PSUM->SBUF evacuation.
  Phase 3: y = out^T.T @ Wfc + bfc + q(residual); layernorm -> x.
"""

import numpy as np

import concourse.bacc as bacc
import concourse.bass as bass
import concourse.mybir as mybir
from concourse.tile import TileContext
from concourse.bass_utils import run_bass_kernel_spmd
from concourse.masks import make_identity

f32 = mybir.dt.float32
f32r = mybir.dt.float32r
f16 = mybir.dt.float16
AF = mybir.ActivationFunctionType
OP = mybir.AluOpType
AX = mybir.AxisListType

P = 128
EPS = 1e-6


def build_nc(S=2048, D=1024, H=16, DK=64, QR=1024):
    """Build + compile the per-core Bass program (SPMD, identical on all cores)."""
    assert H * DK == D and DK == 64
    DC = D // P              # contraction chunks of the model dim
    KC = S // P              # k-row chunks
    HP = H // 2              # head pairs == D//P column chunks
    QW = min(512, QR)        # q-width of one attention unit
    NQH = QR // QW           # attention units per head pair
    NQB = QW // P            # q-blocks per unit
    RB = min(512, S)         # row-block width for phase 1
    VSLOT = DK + 1           # v columns per head + ones column
    ONESW = max(D, RB)

    nc = bacc.Bacc("TRN2", target_bir_lowering=False)

    qs = nc.dram_tensor("qs", [QR, D], f32, kind="ExternalInput")
    kb = nc.dram_tensor("kb", [S, D], f32, kind="ExternalInput")
    vb = nc.dram_tensor("vb", [S, D], f32, kind="ExternalInput")
    Wq = nc.dram_tensor("Wq", [D, D], f32, kind="ExternalInput")
    bq = nc.dram_tensor("bq", [D], f32, kind="ExternalInput")
    Wfc = nc.dram_tensor("Wfc", [D, D], f32, kind="ExternalInput")
    bfc = nc.dram_tensor("bfc", [D], f32, kind="ExternalInput")
    gamma = nc.dram_tensor("gamma", [D], f32, kind="ExternalInput")
    beta = nc.dram_tensor("beta", [D], f32, kind="ExternalInput")
    attn_o = nc.dram_tensor("attn_o", [H, QR, S], f32, kind="ExternalOutput")
    x_o = nc.dram_tensor("x_o", [QR, D], f32, kind="ExternalOutput")

    scale = DK ** -0.5

    with TileContext(nc) as tc:
        with tc.tile_pool(name="persist", bufs=1) as pp:
            id32 = pp.tile([P, P], f32)
            make_identity(nc, id32[:])
            id16 = pp.tile([P, P], f16)
            make_identity(nc, id16[:])
            ones65 = pp.tile([65, ONESW], f32)
            nc.vector.memset(ones65[:], 1.0)

            # per-head unnormalized-free out^T tiles [64, QR] (partition base 0)
            outT = [pp.tile([DK, QR], f16, name=f"outT{h}") for h in range(H)]

            with tc.tile_pool(name="acts", bufs=1) as ap_pool:
                qhT = [ap_pool.tile([P, QR], f16, name=f"qhT{p}") for p in range(HP)]
                khT = [ap_pool.tile([P, S], f16, name=f"khT{p}") for p in range(HP)]
                vh = [ap_pool.tile([P, H, VSLOT], f16, name=f"vh{c}") for c in range(KC)]

                # ------------- Phase 1: transposes + projections -------------
                with tc.tile_pool(name="ph1", bufs=1) as ph1, \
                     tc.tile_pool(name="xload", bufs=(RB // P) + 2) as xl, \
                     tc.tile_pool(name="xt", bufs=1) as xtp, \
                     tc.tile_pool(name="ph1ps", bufs=2, space="PSUM") as ps_t, \
                     tc.tile_pool(name="ph1ps2", bufs=2, space="PSUM") as ps_p:

                    Wq16 = ph1.tile([P, DC, D], f16)
                    with tc.tile_pool(name="wq32", bufs=2) as w32p:
                        for c4 in range(0, D, 256):
                            wq32 = w32p.tile([P, DC, 256], f32, tag="wq32")
                            nc.sync.dma_start(
                                wq32[:],
                                Wq[:, c4:c4 + 256].rearrange("(dc p) c -> p dc c", p=P),
                            )
                            nc.vector.tensor_copy(Wq16[:, :, c4:c4 + 256], wq32[:])
                    bq_f = ph1.tile([1, D], f32)  # free-dim view for bias matmuls
                    nc.sync.dma_start(bq_f[:], bq[None, :])
                    bq16 = ph1.tile([1, D], f16)
                    nc.vector.tensor_copy(bq16[:], bq_f[:])

                    for c in range(KC):
                        nc.vector.memset(vh[c][:, :, DK:VSLOT], 1.0)

                    def transpose_block(src, row0, nrt):
                        """Load nrt*128 rows of src, return X^T tile [P, DC, nrt*128] f16."""
                        xrows = []
                        for rt in range(nrt):
                            xr = xl.tile([P, D], f32, tag="xrow", name="xr")
                            nc.sync.dma_start(
                                xr[:], src[row0 + rt * P: row0 + (rt + 1) * P, :]
                            )
                            xrows.append(xr)
                        xt = xtp.tile([P, DC, nrt * P], f16, tag="xt", name="xt")
                        for dc in range(DC):
                            pt = ps_t.tile([P, nrt * P], f32, tag="pt", name="pt")
                            for rt in range(nrt):
                                nc.tensor.transpose(
                                    pt[:, rt * P:(rt + 1) * P],
                                    xrows[rt][:, dc * P:(dc + 1) * P],
                                    id32[:],
                                )
                            nc.vector.tensor_copy(xt[:, dc, :], pt[:])
                        return xt

                    ones16 = ph1.tile([1, ONESW], f16)
                    nc.vector.memset(ones16[:], 1.0)

                    # q and k: project into transposed layout [cols, rows]
                    for src, dst, n_rows, sc in (
                        (qs, qhT, QR, scale),
                        (kb, khT, S, None),
                    ):
                        rb = min(RB, n_rows)
                        for blk in range(n_rows // rb):
                            xt = transpose_block(src, blk * rb, rb // P)
                            for mc in range(HP):
                                pq = ps_p.tile([P, RB], f32, tag="pq", name="pq")
                                for dc in range(DC):
                                    nc.tensor.matmul(
                                        pq[:, :rb],
                                        Wq16[:, dc, mc * P:(mc + 1) * P],
                                        xt[:, dc, :],
                                        start=(dc == 0), stop=False,
                                    )
                                # + bq[col] broadcast over rows: K=1 matmul
                                nc.tensor.matmul(
                                    pq[:, :rb],
                                    bq16[0:1, mc * P:(mc + 1) * P],
                                    ones16[0:1, :rb],
                                    start=False, stop=True,
                                )
                                if sc is None:
                                    nc.vector.tensor_copy(
                                        dst[mc][:, blk * rb:(blk + 1) * rb], pq[:, :rb]
                                    )
                                else:
                                    nc.vector.tensor_scalar_mul(
                                        dst[mc][:, blk * rb:(blk + 1) * rb],
                                        pq[:, :rb], sc,
                                    )

                    # v: project into natural layout [rows, heads, 64]
                    for blk in range(S // RB):
                        xt = transpose_block(vb, blk * RB, RB // P)
                        for rt in range(RB // P):
                            kc = blk * (RB // P) + rt
                            for w0 in range(0, D, 512):
                                w = min(512, D - w0)
                                nh = w // DK
                                h0 = w0 // DK
                                pv = ps_p.tile([P, RB], f32, tag="pq", name="pv")
                                for dc in range(DC):
                                    nc.tensor.matmul(
                                        pv[:, :w],
                                        xt[:, dc, rt * P:(rt + 1) * P],
                                        Wq16[:, dc, w0:w0 + w],
                                        start=(dc == 0), stop=False,
                                    )
                                nc.tensor.matmul(
                                    pv[:, :w],
                                    ones16[0:1, :P],
                                    bq16[0:1, w0:w0 + w],
                                    start=False, stop=True,
                                )
                                nc.vector.tensor_copy(
                                    vh[kc][:, h0:h0 + nh, 0:DK],
                                    pv[:, :w].rearrange("p (h d) -> p h d", h=nh),
                                )

                # ------------- Phase 2: attention -------------
                units = [(p, qh) for p in range(HP) for qh in range(NQH)]
                TAIL_PACE = 6  # tail steps interleaved per head kc-iteration

                with tc.tile_pool(name="expT", bufs=KC + 8) as ep, \
                     tc.tile_pool(name="pnq", bufs=3) as pnp, \
                     tc.tile_pool(name="smA", bufs=2) as smA, \
                     tc.tile_pool(name="smB", bufs=6) as smB, \
                     tc.tile_pool(name="dscr", bufs=4, space="DRAM") as dsp, \
                     tc.tile_pool(name="psL", bufs=2, space="PSUM") as psL, \
                     tc.tile_pool(name="psO", bufs=2, space="PSUM") as psO, \
                     tc.tile_pool(name="psT", bufs=2, space="PSUM") as psT:

                    state = {}

                    def head_steps(u):
                        """Generator: one yield per kc; computes logitsT, exp,
                        PV accumulation; finishes with recip/outT updates."""
                        p, qh = u
                        q0 = qh * QW
                        chunks = []
                        po = [psO.tile([DK + 1, QW], f32, tag="psO", name=f"po{hi}")
                              for hi in range(2)]
                        for kc in range(KC):
                            pl = psL.tile([P, 2 * QW], f32, tag="psL", name="pl")
                            for hi in range(2):
                                nc.tensor.matmul(
                                    pl[:, hi * QW:(hi + 1) * QW],
                                    khT[p][hi * DK:(hi + 1) * DK, kc * P:(kc + 1) * P],
                                    qhT[p][hi * DK:(hi + 1) * DK, q0:q0 + QW],
                                    start=True, stop=True,
                                )
                            ec = ep.tile([P, 2 * QW], f16, tag="expT", name="ec")
                            nc.scalar.activation(ec[:], pl[:], AF.Exp)
                            chunks.append(ec)
                            for hi in range(2):
                                nc.tensor.matmul(
                                    po[hi][:],
                                    vh[kc][:, 2 * p + hi, :],
                                    ec[:, hi * QW:(hi + 1) * QW],
                                    start=(kc == 0), stop=(kc == KC - 1),
                                )
                            yield
                        recipTs = []
                        for hi in range(2):
                            rc = smA.tile([65, QW], f32, tag="rc", name="rc")
                            nc.vector.reciprocal(rc[64:65, :], po[hi][DK:DK + 1, :])
                            rT = smB.tile([P, NQB], f32, tag="rT", name="rT")
                            rd = dsp.tile([QW], f32, tag="rd", name="rd")
                            nc.gpsimd.dma_start(rd[None, :], rc[64:65, :])
                            nc.gpsimd.dma_start(
                                rT[:], rd[:].rearrange("(qb p) -> p qb", p=P)
                            )
                            recipTs.append(rT)
                            # replicate recip across DK partitions, then
                            # normalize out^T for the fc input
                            pr = psT.tile([DK, QW], f32, tag="psT", name="pr")
                            nc.tensor.matmul(
                                pr[:],
                                ones65[64:65, 0:DK],
                                rc[64:65, :],
                                start=True, stop=True,
                            )
                            rcr = smA.tile([DK, QW], f32, tag="rcr", name="rcr")
                            nc.vector.tensor_copy(rcr[:], pr[:])
                            nc.vector.tensor_tensor(
                                outT[2 * p + hi][:, q0:q0 + QW],
                                po[hi][0:DK, :], rcr[:], OP.mult,
                            )
                        state[u] = (chunks, recipTs)

                    def tail_steps(u):
                        """Generator: one yield per (hi, qb, kc-group) batch;
                        PE-transposes exp chunks, normalizes, DMAs attn out."""
                        p, qh = u
                        chunks, recipTs = state.pop(u)
                        for hi in range(2):
                            hg = 2 * p + hi
                            for qb in range(NQB):
                                r0 = qh * QW + qb * P
                                for kg0 in range(0, KC, 4):
                                    kg = min(4, KC - kg0)
                                    pt = psT.tile([P, 4 * P], f16, tag="psT", name="pt")
                                    for kci in range(kg):
                                        nc.tensor.transpose(
                                            pt[:, kci * P:(kci + 1) * P],
                                            chunks[kg0 + kci][
                                                :, hi * QW + qb * P: hi * QW + (qb + 1) * P
                                            ],
                                            id16[:],
                                        )
                                    pnq = pnp.tile([P, 4 * P], f32, tag="pnq", name="pnq")
                                    nc.vector.tensor_scalar(
                                        pnq[:, :kg * P], pt[:, :kg * P],
                                        recipTs[hi][:, qb:qb + 1], None, OP.mult,
                                    )
                                    nc.sync.dma_start(
                                        attn_o[hg, r0:r0 + P, kg0 * P:(kg0 + kg) * P],
                                        pnq[:, :kg * P],
                                    )
                                    yield

                    pending_tail = None
                    for u in units:
                        hs = head_steps(u)
                        for _ in hs:
                            if pending_tail is not None:
                                for _ in range(TAIL_PACE):
                                    if next(pending_tail, StopIteration) is StopIteration:
                                        pending_tail = None
                                        break
                        pending_tail = tail_steps(u)
                    for _ in pending_tail:
                        pass

            # ------------- Phase 3: fc + residual + layernorm -------------
            with tc.tile_pool(name="ph3", bufs=1) as ph3, \
                 tc.tile_pool(name="ph3w", bufs=2) as p3w, \
                 tc.tile_pool(name="psY", bufs=2, space="PSUM") as psY, \
                 tc.tile_pool(name="psB", bufs=1, space="PSUM") as psB:

                # Wfc in [64 (within-head dim), H, D] layout, fp16
                Wfc16 = ph3.tile([DK, H, D], f16)
                with tc.tile_pool(name="wfc32", bufs=2) as w32p:
                    for h in range(H):
                        wf = w32p.tile([DK, D], f32, tag="wf", name="wf")
                        nc.sync.dma_start(wf[:], Wfc[h * DK:(h + 1) * DK, :])
                        nc.vector.tensor_copy(Wfc16[:, h, :], wf[:])

                # replicate bfc/gamma/beta across partitions via K=1 matmul
                vec_f = ph3.tile([1, D], f32)
                reps = []
                for name, src in (("bfc", bfc), ("gam", gamma), ("bet", beta)):
                    nc.sync.dma_start(vec_f[:], src[None, :])
                    pb = psB.tile([P, D], f32, tag="psB", name="pb")
                    for w0 in range(0, D, 512):
                        w = min(512, D - w0)
                        nc.tensor.matmul(
                            pb[:, w0:w0 + w], ones65[0:1, 0:P],
                            vec_f[0:1, w0:w0 + w], start=True, stop=True,
                        )
                    rep = ph3.tile([P, D], f32, name=f"rep_{name}")
                    nc.vector.tensor_copy(rep[:], pb[:])
                    reps.append(rep)
                bfc_r, gam_r, bet_r = reps

                inv_d = 1.0 / D
                for qt in range(QR // P):
                    py = psY.tile([P, D], f32, tag="py", name="py")
                    for w0 in range(0, D, 512):
                        w = min(512, D - w0)
                        for h in range(H):
                            nc.tensor.matmul(
                                py[:, w0:w0 + w],
                                outT[h][:, qt * P:(qt + 1) * P],
                                Wfc16[:, h, w0:w0 + w],
                                start=(h == 0), stop=(h == H - 1),
                            )
                    res = p3w.tile([P, D], f32, tag="res", name="res")
                    nc.sync.dma_start(res[:], qs[qt * P:(qt + 1) * P, :])
                    xt = p3w.tile([P, D], f32, tag="xt", name="xt")
                    nc.vector.tensor_tensor(xt[:], py[:], res[:], OP.add)
                    nc.vector.tensor_tensor(xt[:], xt[:], bfc_r[:], OP.add)
                    # layernorm
                    sm = p3w.tile([P, 1], f32, tag="sm", name="sm")
                    nc.vector.reduce_sum(sm[:], xt[:], axis=AX.X)
                    mu = p3w.tile([P, 1], f32, tag="mu", name="mu")
                    nc.vector.tensor_scalar_mul(mu[:], sm[:], inv_d)
                    sq = p3w.tile([P, D], f32, tag="sq", name="sq")
                    nc.vector.tensor_tensor(sq[:], xt[:], xt[:], OP.mult)
                    s2 = p3w.tile([P, 1], f32, tag="s2", name="s2")
                    nc.vector.reduce_sum(s2[:], sq[:], axis=AX.X)
                    var = p3w.tile([P, 1], f32, tag="var", name="var")
                    nc.vector.tensor_scalar_mul(var[:], s2[:], inv_d)
                    mu2 = p3w.tile([P, 1], f32, tag="mu2", name="mu2")
                    nc.vector.tensor_tensor(mu2[:], mu[:], mu[:], OP.mult)
                    nc.vector.tensor_tensor(var[:], var[:], mu2[:], OP.subtract)
                    nc.vector.tensor_scalar_add(var[:], var[:], EPS)
                    sd = p3w.tile([P, 1], f32, tag="sd", name="sd")
                    nc.scalar.activation(sd[:], var[:], AF.Sqrt)
                    rstd = p3w.tile([P, 1], f32, tag="rstd", name="rstd")
                    nc.vector.reciprocal(rstd[:], sd[:])
                    xh = p3w.tile([P, D], f32, tag="xh", name="xh")
                    nc.vector.tensor_scalar(
                        xh[:], xt[:], mu[:], rstd[:], OP.subtract, OP.mult
                    )
                    nc.vector.tensor_tensor(xh[:], xh[:], gam_r[:], OP.mult)
                    nc.vector.tensor_tensor(xh[:], xh[:], bet_r[:], OP.add)
                    nc.sync.dma_start(x_o[qt * P:(qt + 1) * P, :], xh[:])

    nc.compile()
    return nc


_NC = None
_LAST_RES = None


def _get_nc():
    global _NC
    if _NC is None:
        _NC = build_nc()
    return _NC


def kernel(q, k, v, Wq, bq, Wfc, bfc, gamma, beta):
    B, S, D = 4, 2048, 1024
    H = 16
    QR = S * B // 8  # 1024 query rows per core

    q = np.ascontiguousarray(np.asarray(q, dtype=np.float32))
    k = np.ascontiguousarray(np.asarray(k, dtype=np.float32))
    v = np.ascontiguousarray(np.asarray(v, dtype=np.float32))
    common = {
        "Wq": np.ascontiguousarray(np.asarray(Wq, np.float32)),
        "bq": np.ascontiguousarray(np.asarray(bq, np.float32)),
        "Wfc": np.ascontiguousarray(np.asarray(Wfc, np.float32)),
        "bfc": np.ascontiguousarray(np.asarray(bfc, np.float32)),
        "gamma": np.ascontiguousarray(np.asarray(gamma, np.float32)),
        "beta": np.ascontiguousarray(np.asarray(beta, np.float32)),
    }
    in_maps = []
    for c in range(8):
        b, half = c // 2, c % 2
        in_maps.append({
            "qs": np.ascontiguousarray(q[b, half * QR:(half + 1) * QR]),
            "kb": k[b],
            "vb": v[b],
            **common,
        })

    nc = _get_nc()
    res = run_bass_kernel_spmd(nc, in_maps, list(range(8)))
    global _LAST_RES
    _LAST_RES = res

    x = np.empty((B, S, D), np.float32)
    attn = np.empty((B, H, S, S), np.float32)
    for c in range(8):
        b, half = c // 2, c % 2
        r = res.results[c]
        x[b, half * QR:(half + 1) * QR] = r["x_o"]
        attn[b, :, half * QR:(half + 1) * QR, :] = r["attn_o"]
    return (x, attn)
